# revision 18
# baseline (speedup 1.0000x reference)
"""Multi-head self-attention with RoPE on 8 Trainium2 NeuronCores.

Full inputs in, full output out. Sharding: batch (2) x head-groups (4 heads
per core). Each core computes qkv projections for its heads, RoPE, full
softmax(QK^T)V, and a combined (both head-pairs) partial output projection;
host sums the 4 partials per batch element and adds b_out.

v2: row-tiled score matmuls — each head's QK^T contracts only 64 dims, so
the two heads of a pair run CONCURRENTLY in disjoint PE row-groups
(tile_position (0,0)/(64,0)), halving score PE time. Attention runs in
512-query "eighths" so PSUM fits: score groups [128,1024] (A|B) x2 bufs
(4 banks) + 2 pv accumulators (2 banks) + aux ring (2 banks) = 8 banks.
Emission is software-pipelined per step s: pv(s-1) | score(s+1)/exp(s+1) |
deadline-scheduled micro-units (projections, v, norm, outproj).

Problem shape: B=2, T=2048, D=1024, H=16, HD=64 (hardcoded).
"""

import numpy as np
from contextlib import ExitStack

import ml_dtypes
import concourse.bass as bass
import concourse.mybir as mybir
import concourse.tile as tile
from concourse import bass_utils

B, T, D, H = 2, 2048, 1024, 16
HD = 64          # head dim
HL = 4           # heads per core
N_CORES = 8
ROPE_BASE = 10000.0

F32 = mybir.dt.float32
F32R = mybir.dt.float32r
BF16 = mybir.dt.bfloat16
BFNP = ml_dtypes.bfloat16

Exp = mybir.ActivationFunctionType.Exp

NT = T // 128     # 16 key tiles
NK = D // 128     # 8 contraction chunks
SC = HD ** -0.5
NSTEP = 128       # 2 pairs x 4 eighths x 16 key tiles

# results of the last run (for test harness introspection)
LAST_RESULTS = None
TRACE = False


def _split_excess_waits(nc, cap=1):
    """walrus in this env rejects >1 sync-wait per instruction; split extras
    onto single-wait NoOps on the same engine queue."""
    n = 0
    for f in nc.m.functions:
        for bb in f.blocks:
            insts = bb.instructions
            if not any(
                i.sync_info is not None and len(i.sync_info.on_wait) > cap
                for i in insts
            ):
                continue
            out = []
            for inst in insts:
                si = inst.sync_info
                waits = list(si.on_wait) if si is not None else []
                if len(waits) > cap:
                    extra, keep = waits[:-cap], waits[-cap:]
                    for k, w in enumerate(extra):
                        nop = mybir.InstNoOp(
                            name=f"{inst.name}-ws{k}",
                            engine=inst.engine,
                            sync_info=mybir.SyncInfo(on_wait=[w], on_update=[]),
                            bass_nofuse=True,
                        )
                        nc.register_instruction(nop)
                        out.append(nop)
                        n += 1
                    inst.sync_info = mybir.SyncInfo(
                        on_wait=keep, on_update=list(si.on_update)
                    )
                out.append(inst)
            bb.instructions = out
    return n


def _build_bass(with_qkv_bias, with_v_bias):
    nc = bass.Bass("TRN2", target_bir_lowering=False, debug=False, num_devices=1)

    # ---- DRAM I/O ----
    d_xT = nc.dram_tensor("xT", [D, T], BF16, kind="ExternalInput").ap()
    d_wqk = nc.dram_tensor("wqk", [D, 4 * 128], BF16, kind="ExternalInput").ap()
    d_wv = nc.dram_tensor("wv", [D, HL * (HD + 1)], BF16, kind="ExternalInput").ap()
    d_bqk = nc.dram_tensor("bqk", [1, 4 * 128], BF16, kind="ExternalInput").ap()
    d_bv = nc.dram_tensor("bv", [1, HL * (HD + 1)], BF16, kind="ExternalInput").ap()
    d_ones = nc.dram_tensor("ones", [1, 512], BF16, kind="ExternalInput").ap()
    d_cos = nc.dram_tensor("cos2", [128, T], F32, kind="ExternalInput").ap()
    d_sin = nc.dram_tensor("sin2", [128, T], F32, kind="ExternalInput").ap()
    d_rT = nc.dram_tensor("rT", [128, 128], BF16, kind="ExternalInput").ap()
    d_ind = nc.dram_tensor("ind", [2, 128], BF16, kind="ExternalInput").ap()
    d_amask = nc.dram_tensor("amask", [128, NT], F32, kind="ExternalInput").ap()
    d_wo = nc.dram_tensor("wo", [2 * 128, D], BF16, kind="ExternalInput").ap()
    d_out = nc.dram_tensor("out_part", [T, D], BF16, kind="ExternalOutput").ap()

    with tile.TileContext(nc) as tc, ExitStack() as ctx:
        pool = lambda name, bufs: ctx.enter_context(tc.tile_pool(name=name, bufs=bufs))
        psum = lambda name, bufs: ctx.enter_context(
            tc.tile_pool(name=name, bufs=bufs, space="PSUM")
        )

        p_const = pool("const", 1)
        p_xt = pool("xt", 1)
        p_w = pool("w", 1)
        p_wv = pool("wv", 1)
        p_cs = pool("cs", 1)
        p_tmp = pool("tmp", 2)
        p_qk = pool("qk", 1)
        p_v = pool("v", NT)
        p_e = pool("e", 4)
        p_at = pool("at", 4)
        p_an = pool("an", 2)
        p_fin = pool("fin", 2)

        ps_s = psum("ps_s", 2)      # score groups [128,1024] -> 4 banks
        ps_pv = psum("ps_pv", 1)    # pvA+pvB [65,512] -> 2 banks
        ps_aux = psum("ps_aux", 2)  # [128,512] ring -> 2 banks

        # ---- input loads ----
        # x token-quarter 0 rides the sync ring in per-chunk descriptors
        # interleaved with wqk chunks, so chunk-k matmuls start as soon as
        # pair k lands. Tables split across scalar+vector rings.
        xt_all = p_xt.tile([128, NK * T], BF16, tag="xt", bufs=1, name="xt_all")
        wqk_all = p_w.tile([128, NK * 512], BF16, tag="wqk", bufs=1,
                           name="wqk_all")
        xt3 = xt_all[:].rearrange("p (c w) -> p c w", c=NK)
        xsrc = d_xT[:].rearrange("(c p) w -> p c w", p=128)
        # need-ordered: the q/k weight groups for pair 0 and x token-quarter
        # 0 gate the prologue matmuls; everything else follows.
        wqk3 = wqk_all[:].rearrange("p (c w) -> p c w", c=NK)
        wsrc = d_wqk[:].rearrange("(c p) w -> p c w", p=128)
        with tc.high_priority():
            for g in (0, 2):
                nc.sync.dma_start(wqk3[:, :, g * 128:(g + 1) * 128],
                                  wsrc[:, :, g * 128:(g + 1) * 128])
            nc.sync.dma_start(xt3[:, 0:4, 0:512], xsrc[:, 0:4, 0:512])
            nc.sync.dma_start(xt3[:, 4:8, 0:512], xsrc[:, 4:8, 0:512])
        nc.sync.dma_start(xt3[:, :, 1536:2048], xsrc[:, :, 1536:2048])
        for g in (1, 3):
            nc.sync.dma_start(wqk3[:, :, g * 128:(g + 1) * 128],
                              wsrc[:, :, g * 128:(g + 1) * 128])

        def xt(k):
            return xt_all[:, k * T:(k + 1) * T]

        def wqk_sb(k):
            return wqk_all[:, k * 512:(k + 1) * 512]

        # scalar ring: rope tables / mask / v weights (needed early)
        t_rT = p_const.tile([128, 128], BF16, tag="rT")
        nc.scalar.dma_start(t_rT[:], d_rT[:])
        t_warm = p_const.tile([1, 16], BF16, tag="warmrow")
        nc.scalar.activation(t_warm[:], t_rT[0:1, 0:16], Exp)
        t_cos = p_cs.tile([128, T], F32, tag="cos")
        t_sin = p_cs.tile([128, T], F32, tag="sin")
        nc.gpsimd.dma_start(t_sin[:, 0:1024], d_sin[:, 0:1024])
        nc.gpsimd.dma_start(t_cos[:, 0:1024], d_cos[:, 0:1024])
        nc.gpsimd.dma_start(xt3[:, :, 1024:1536], xsrc[:, :, 1024:1536])
        nc.gpsimd.dma_start(t_sin[:, 1024:T], d_sin[:, 1024:T])
        nc.gpsimd.dma_start(t_cos[:, 1024:T], d_cos[:, 1024:T])
        t_amask = p_const.tile([128, NT], F32, tag="amask")
        nc.scalar.dma_start(t_amask[:], d_amask[:])
        wv_all = p_wv.tile([128, NK * 260], BF16, tag="wv", bufs=1,
                           name="wv_all")
        nc.scalar.dma_start(
            wv_all[:].rearrange("p (c w) -> p c w", c=NK),
            d_wv[:].rearrange("(c p) w -> p c w", p=128),
        )
        nc.scalar.dma_start(xt3[:, :, 512:1024], xsrc[:, :, 512:1024])
        t_ones = p_const.tile([1, 512], BF16, tag="ones")
        nc.scalar.dma_start(t_ones[:], d_ones[:])
        t_bqk = p_const.tile([1, 4 * 128], BF16, tag="bqk")
        nc.scalar.dma_start(t_bqk[:], d_bqk[:])
        t_bv = p_const.tile([1, HL * (HD + 1)], BF16, tag="bv")
        nc.scalar.dma_start(t_bv[:], d_bv[:])

        t_indA = p_const.tile([1, 128], BF16, tag="indA")
        nc.gpsimd.dma_start(t_indA[:], d_ind[0:1, :])
        t_indB = p_const.tile([1, 128], BF16, tag="indB")
        nc.gpsimd.dma_start(t_indB[:], d_ind[1:2, :])

        def wv_sb(k):
            return wv_all[:, k * 260:(k + 1) * 260]

        # out-proj weights: not needed until late; sync ring after x
        wo_sb = []
        for c2 in range(2):
            wt = p_fin.tile([128, D], BF16, tag="wo", name="wo_t")
            nc.sync.dma_start(wt[:], d_wo[c2 * 128:(c2 + 1) * 128, :])
            wo_sb.append(wt)

        # ---- persistent q/k tiles (head A dims on partitions 0:64,
        # head B on 64:128 — no zero padding needed with row tiling) ----
        qc = [p_qk.tile([128, T], BF16, tag=f"qc{p}", name="qc_t")
              for p in range(2)]
        kt = [p_qk.tile([128, T], BF16, tag=f"k{p}", name="k_t")
              for p in range(2)]

        v_sb = [None] * NT
        at_t = [None] * 4   # (pair, hh) -> [65, T] f32 unnormalized attn
        an_t = [None] * 2   # pair -> [128, T] bf16 normalized attn
        for p in range(2):
            an_t[p] = p_an.tile([128, T], BF16, tag="an", name="an_t")

        # ---- rope / projection emitters ----
        def emit_proj_mms(acc, c2, sl):
            for k in range(NK):
                nc.tensor.matmul(
                    acc,
                    wqk_sb(k)[:, c2 * 128:(c2 + 1) * 128],
                    xt(k)[:, sl],
                    start=(k == 0),
                    stop=(not with_qkv_bias and k == NK - 1),
                    skip_group_check=True,
                )
            if with_qkv_bias:
                nc.tensor.matmul(
                    acc,
                    t_bqk[:, c2 * 128:(c2 + 1) * 128],
                    t_ones[:, 0:512],
                    start=False,
                    stop=True,
                    skip_group_check=True,
                )

        def emit_rope(acc, qi, pair, is_k, rot_ring="aux"):
            """RoPE: roped = raw*cos + R @ (raw*sin); store q/k bf16."""
            sl = slice(qi * 512, (qi + 1) * 512)
            u = p_tmp.tile([128, 512], BF16, tag="u", name="u_t")
            nc.vector.tensor_mul(u[:], acc, t_sin[:, sl])
            if rot_ring == "pv":
                rot = ps_pv.tile([128, 512], F32, tag="pvA", name="rot")
            else:
                rot = ps_aux.tile([128, 512], F32, tag="aux", name="rot")
            nc.tensor.matmul(rot[:], t_rT[:], u[:], start=True, stop=True,
                             skip_group_check=True)
            c_sb = p_tmp.tile([128, 512], F32, tag="c", name="c_t")
            nc.vector.tensor_mul(c_sb[:], acc, t_cos[:, sl])
            dst = qc[pair] if not is_k else kt[pair]
            nc.vector.tensor_add(dst[:, sl], c_sb[:], rot[:])

        def emit_v_fin(j, acc):
            av = acc[:, 0:HL * (HD + 1)]
            vt = p_v.tile([128, HL * (HD + 1)], BF16, tag="v", name="v_t")
            nc.vector.tensor_copy(vt[:], av)
            if not with_v_bias:
                ones_cols = vt[:].rearrange("p (h c) -> p h c", h=HL)[:, :, HD:HD + 1]
                nc.gpsimd.memset(ones_cols, 1.0)
            v_sb[j] = vt

        # ---- micro-unit builders (each callable ~1-2 MMs of PE work) ----
        def proj_unit_micros(c2, qi, pair, is_k):
            """Projection quarter as 5 micros: 4x 2-chunk MMs + rope."""
            box = [None]

            def alloc_and_mm(kk):
                def f():
                    if box[0] is None:
                        box[0] = ps_aux.tile([128, 512], F32, tag="aux",
                                             name="acc")
                    acc = box[0]
                    sl = slice(qi * 512, (qi + 1) * 512)
                    for k in (kk, kk + 1):
                        nc.tensor.matmul(
                            acc[:], wqk_sb(k)[:, c2 * 128:(c2 + 1) * 128],
                            xt(k)[:, sl], start=(k == 0),
                            stop=(not with_qkv_bias and k == NK - 1),
                            skip_group_check=True,
                        )
                    if with_qkv_bias and kk == NK - 2:
                        nc.tensor.matmul(
                            acc[:], t_bqk[:, c2 * 128:(c2 + 1) * 128],
                            t_ones[:, 0:512], start=False, stop=True,
                            skip_group_check=True,
                        )
                return f

            def rope_f():
                emit_rope(box[0][:], qi, pair, is_k)

            return [alloc_and_mm(k) for k in range(0, NK, 2)] + [rope_f]

        def v_unit_micros(j):
            """V projection for key tile j as 5 micros."""
            box = [None]

            def mm(kk):
                def f():
                    if box[0] is None:
                        box[0] = ps_aux.tile([128, 512], F32, tag="aux",
                                             name="vacc")
                    av = box[0][:, 0:HL * (HD + 1)]
                    for k in (kk, kk + 1):
                        nc.tensor.matmul(
                            av, xt(k)[:, j * 128:(j + 1) * 128], wv_sb(k)[:],
                            start=(k == 0),
                            stop=(not with_v_bias and k == NK - 1),
                            skip_group_check=True,
                        )
                    if with_v_bias and kk == NK - 2:
                        nc.tensor.matmul(av, t_ones[:, 0:128], t_bv[:],
                                         start=False, stop=True,
                                         skip_group_check=True)
                return f

            def fin():
                emit_v_fin(j, box[0])

            return [mm(k) for k in range(0, NK, 2)] + [fin]

        # ---- normalization ----
        norm_state = {}

        def emit_norm_head(pair, e8):
            """after eighth (pair, e8): denominator reciprocal round-trip.
            Gather the pv PSUM denominator rows into a partition-spread
            [128,8] layout, reciprocal there (vectorized), scatter back to
            flat bf16 rows for the pb broadcast matmuls."""
            sums = p_fin.tile([128, 8], F32, tag="sums", bufs=2, name="sums_t")
            rec = p_fin.tile([128, 8], BF16, tag="rec", bufs=2, name="rec_t")
            rrA = p_fin.tile([1, 512], BF16, tag="rrA", bufs=2, name="rrA_t")
            rrB = p_fin.tile([1, 512], BF16, tag="rrB", bufs=2, name="rrB_t")
            ar = p_fin.tile([128, 512], F32, tag="ar", bufs=2, name="ar_t")
            sl = slice(e8 * 512, (e8 + 1) * 512)
            for hh in range(2):
                nc.sync.dma_start(
                    sums[:, hh * 4:(hh + 1) * 4],
                    at_t[2 * pair + hh][HD:HD + 1, sl].rearrange(
                        "o (p c) -> o p c", p=128))
            with nc.allow_low_precision(reason="1/denominator in bf16"):
                nc.vector.reciprocal(rec[:], sums[:])
            nc.sync.dma_start(rrA[:].rearrange("o (p c) -> o p c", p=128),
                              rec[:, 0:4])
            nc.sync.dma_start(rrB[:].rearrange("o (p c) -> o p c", p=128),
                              rec[:, 4:8])
            norm_state[(pair, e8)] = ((rrA, rrB), ar)

        def emit_ar(pair, e8):
            sl = slice(e8 * 512, (e8 + 1) * 512)
            _, ar = norm_state[(pair, e8)]
            nc.sync.dma_start(ar[HD:128, :], at_t[2 * pair + 1][0:HD, sl])

        def fin_micros(pair, e8):
            """pb broadcast matmuls + an multiplies (lag-scheduled)."""
            sl = slice(e8 * 512, (e8 + 1) * 512)
            box = [None]

            def pb_f():
                (rrA, rrB), ar = norm_state[(pair, e8)]
                pb = ps_aux.tile([128, 512], F32, tag="aux", name="pb")
                nc.tensor.matmul(pb[:], t_indA[:], rrA[:],
                                 start=True, stop=False, skip_group_check=True)
                nc.tensor.matmul(pb[:], t_indB[:], rrB[:],
                                 start=False, stop=True, skip_group_check=True)
                box[0] = pb

            def an_f():
                _, ar = norm_state.pop((pair, e8))
                pb = box[0]
                nc.vector.tensor_mul(an_t[pair][0:HD, sl],
                                     at_t[2 * pair][0:HD, sl], pb[0:HD, :])
                nc.vector.tensor_mul(an_t[pair][HD:128, sl],
                                     ar[HD:128, :], pb[HD:128, :])

            return [pb_f, an_f]

        def outproj_micros(t, tail=False):
            """output projection for token tile t, both pairs accumulated."""
            off = (t % NT) * 128
            osb = p_fin.tile([128, D], BF16, tag="osb", bufs=4, name="osb_t")

            def half(n5):
                def f():
                    s5 = slice(n5 * 512, (n5 + 1) * 512)
                    pp = ps_aux.tile([128, 512], F32, tag="aux", name="pp")
                    nc.tensor.matmul(pp[:], an_t[0][:, off:off + 128],
                                     wo_sb[0][:, s5],
                                     start=True, stop=False,
                                     skip_group_check=True)
                    nc.tensor.matmul(pp[:], an_t[1][:, off:off + 128],
                                     wo_sb[1][:, s5],
                                     start=False, stop=True,
                                     skip_group_check=True)
                    if tail and n5 == 1:
                        nc.scalar.copy(osb[:, s5], pp[:])
                    else:
                        nc.vector.tensor_copy(osb[:, s5], pp[:])
                return f

            def dma_f():
                nc.gpsimd.dma_start(d_out[t * 128:(t + 1) * 128, :], osb[:])

            return [half(0), half(1), dma_f]

        # ---- static micro-unit schedule: step -> [callables] ----
        # NOTE: emission order IS logical program order for the dependency
        # tracker — every unit's final write must be emitted strictly before
        # its first reader. score(s+1) is emitted at the top of step s, and
        # pv(s)/v_sb[jb] at the bottom, which sets the deadlines below.
        sched = [[] for _ in range(NSTEP + 40)]

        def add(step, micros, stride=1):
            s = step
            for m in micros:
                sched[min(max(s, 0), len(sched) - 1)].append(m)
                s += stride

        # v projections: v(j) must be fully emitted before pv(j) at step j
        for j in range(2, NT):
            if j <= 6:
                add(max(0, j - 5), v_unit_micros(j), stride=0)
            else:
                add(j - 6, v_unit_micros(j), stride=1)
        # deadlines: score(s) is emitted at the top of step s-2, so a
        # quarter read from step X must have its rope emitted by step X-3.
        add(0, proj_unit_micros(2, 1, 0, True), stride=0)    # rope @0 <= 1
        add(0, proj_unit_micros(2, 2, 0, True), stride=1)    # rope @5 <= 5
        add(4, proj_unit_micros(2, 3, 0, True), stride=1)    # rope @9 <= 9
        add(8, proj_unit_micros(0, 1, 0, False), stride=1)   # rope @13 <= 13
        add(20, proj_unit_micros(0, 2, 0, False), stride=1)  # rope @25 <= 29
        add(26, proj_unit_micros(3, 0, 1, True), stride=1)   # rope @31 <= 61
        add(32, proj_unit_micros(3, 1, 1, True), stride=1)   # rope @37 <= 65
        add(38, proj_unit_micros(0, 3, 0, False), stride=1)  # rope @43 <= 45
        add(44, proj_unit_micros(3, 2, 1, True), stride=1)   # rope @49 <= 69
        add(50, proj_unit_micros(3, 3, 1, True), stride=1)   # rope @55 <= 73
        add(56, proj_unit_micros(1, 0, 1, False), stride=1)  # rope @61 <= 61
        add(66, proj_unit_micros(1, 1, 1, False), stride=1)  # rope @71 <= 77
        add(78, proj_unit_micros(1, 2, 1, False), stride=1)  # rope @83 <= 93
        add(94, proj_unit_micros(1, 3, 1, False), stride=1)  # rope @99 <= 109
        # norm fins: ~4.5us after each eighth ends (DMA round-trip hiding)
        for p in range(2):
            for e8 in range(4):
                add(64 * p + 16 * e8 + 20, fin_micros(p, e8))
        # outproj: tiles 0..7 interleave after fin(1, g); tiles 8..11 fill
        # the tail's norm-chain latency gap; 12..15 after fin(1,3)
        for g in range(2):
            for i in range(4):
                t = 4 * g + i
                add(64 + 16 * g + 22 + 2 * i, outproj_micros(t))
        for i in range(4):
            add(NSTEP + i, outproj_micros(8 + i, tail=True))
        for i in range(4):
            add(NSTEP + 8 + i, outproj_micros(12 + i, tail=True))

        # ---- prologue ----
        # warm-up: junk matmuls bridge the DMA wait so HAM reaches 8/8
        # before real work, and a junk activation preloads the exp table.
        warm_ps = ps_aux.tile([128, 512], F32, tag="aux", name="warm_ps")
        for _ in range(55):
            nc.tensor.matmul(warm_ps[:, 0:128], t_rT[:], t_rT[:],
                             start=True, stop=True, skip_group_check=True)
        # k0-q0 + q0-e0 accs in a borrowed score tile
        big = ps_s.tile([128, 1024], F32, tag="s", name="acc_big")
        aK = big[:, 0:512]
        aQ = big[:, 512:1024]
        for k in range(NK):
            last = not with_qkv_bias and k == NK - 1
            nc.tensor.matmul(aK, wqk_sb(k)[:, 2 * 128:3 * 128],
                             xt(k)[:, 0:512], start=(k == 0), stop=last,
                             skip_group_check=True)
            nc.tensor.matmul(aQ, wqk_sb(k)[:, 0:128],
                             xt(k)[:, 0:512], start=(k == 0), stop=last,
                             skip_group_check=True)
        if with_qkv_bias:
            nc.tensor.matmul(aK, t_bqk[:, 2 * 128:3 * 128], t_ones[:, 0:512],
                             start=False, stop=True, skip_group_check=True)
            nc.tensor.matmul(aQ, t_bqk[:, 0:128], t_ones[:, 0:512],
                             start=False, stop=True, skip_group_check=True)
        emit_rope(aK, 0, 0, True, rot_ring="pv")
        emit_rope(aQ, 0, 0, False, rot_ring="pv")
        # v(0), v(1) through the aux ring
        for j in range(2):
            for m in v_unit_micros(j):
                m()

        # ---- attention steps ----
        def step_info(s):
            pair = s // 64
            e8 = (s // 16) % 4
            jb = s % 16
            return pair, e8, jb

        score_of = {}
        e_of = {}
        pv_box = [None, None]

        def emit_score(s):
            pair, e8, jb = step_info(s)
            sl = slice(e8 * 512, (e8 + 1) * 512)
            sAB = ps_s.tile([128, 1024], F32, tag="s", name="sAB")
            ch = slice(jb * 128, (jb + 1) * 128)
            nc.tensor.matmul(sAB[:, 0:512], kt[pair][0:64, ch],
                             qc[pair][0:64, sl], start=True, stop=True,
                             skip_group_check=True, tile_position=(0, 0))
            nc.tensor.matmul(sAB[:, 512:1024], kt[pair][64:128, ch],
                             qc[pair][64:128, sl], start=True, stop=True,
                             skip_group_check=True, tile_position=(64, 0))
            score_of[s] = sAB

        def emit_exp(s):
            pair, e8, jb = step_info(s)
            sAB = score_of.pop(s)
            e = p_e.tile([128, 1024], BF16, tag="e", name="e_t")
            nc.scalar.activation(e[:], sAB[:], Exp,
                                 bias=t_amask[:, jb:jb + 1], scale=SC)
            e_of[s] = e

        def emit_pv(s):
            pair, e8, jb = step_info(s)
            if jb == 0:
                pv_box[0] = ps_pv.tile([HD + 1, 512], F32, tag="pvA",
                                       name="pvA_t")
                pv_box[1] = ps_pv.tile([HD + 1, 512], F32, tag="pvB",
                                       name="pvB_t")
            e = e_of.pop(s)
            h0 = 2 * pair
            nc.tensor.matmul(pv_box[0][:],
                             v_sb[jb][:, h0 * (HD + 1):(h0 + 1) * (HD + 1)],
                             e[:, 0:512], start=(jb == 0), stop=(jb == NT - 1),
                             skip_group_check=True)
            nc.tensor.matmul(pv_box[1][:],
                             v_sb[jb][:, (h0 + 1) * (HD + 1):(h0 + 2) * (HD + 1)],
                             e[:, 512:1024], start=(jb == 0), stop=(jb == NT - 1),
                             skip_group_check=True)

        def emit_atcopy(pair, e8):
            sl = slice(e8 * 512, (e8 + 1) * 512)
            for hh in range(2):
                if at_t[2 * pair + hh] is None:
                    at_t[2 * pair + hh] = p_at.tile([HD + 1, T], F32, tag="aT",
                                                    name="at_t")
            for hh in range(2):
                nc.vector.tensor_copy(at_t[2 * pair + hh][:, sl],
                                      pv_box[hh][:])
            emit_norm_head(pair, e8)
            emit_ar(pair, e8)

        emit_score(0)
        emit_exp(0)
        emit_score(1)
        emit_exp(1)
        for s in range(NSTEP):
            if s + 2 < NSTEP:
                emit_score(s + 2)
                emit_exp(s + 2)
            for m in sched[s]:
                m()
            if s >= 1:
                emit_pv(s - 1)
                pair, e8, jb = step_info(s - 1)
                if jb == NT - 1:
                    emit_atcopy(pair, e8)
        emit_pv(NSTEP - 1)
        emit_atcopy(1, 3)

        # ---- tail: remaining scheduled micros in order ----
        for s in range(NSTEP, len(sched)):
            for m in sched[s]:
                m()

    _split_excess_waits(nc)
    return nc


_NC_CACHE = {}


def _rope_tables():
    inv_freq = (1.0 / (ROPE_BASE ** (np.arange(0, HD, 2, dtype=np.float32) / HD))
                ).astype(np.float32)
    t = np.arange(T, dtype=np.float32)
    freqs = np.einsum("t,f->tf", t, inv_freq).astype(np.float32)  # (T, HD/2)
    emb = np.concatenate([freqs, freqs], axis=-1)                  # (T, HD)
    cosT = np.cos(emb).astype(np.float32).T                        # (HD, T)
    sinT = np.sin(emb).astype(np.float32).T
    cosT = np.ascontiguousarray(np.concatenate([cosT, cosT], axis=0))
    sinT = np.ascontiguousarray(np.concatenate([sinT, sinT], axis=0))
    return cosT, sinT


def _rot_matrix():
    r = np.zeros((128, 128), dtype=np.float32)
    for p0 in (0, 64):
        for d in range(32):
            r[p0 + d, p0 + 32 + d] = -1.0
            r[p0 + 32 + d, p0 + d] = 1.0
    return np.ascontiguousarray(r.T)


def kernel(x, W_qkv, b_qkv, W_out, b_out, padding_mask):
    global _NC_CACHE, LAST_RESULTS
    x = np.asarray(x, dtype=np.float32)
    W_qkv = np.asarray(W_qkv, dtype=np.float32)
    b_qkv = np.asarray(b_qkv, dtype=np.float32)
    W_out = np.asarray(W_out, dtype=np.float32)
    b_out = np.asarray(b_out, dtype=np.float32)
    padding_mask = np.asarray(padding_mask)

    with_qkv_bias = bool(np.any(b_qkv[:2 * D]))
    with_v_bias = bool(np.any(b_qkv[2 * D:]))
    key = (with_qkv_bias, with_v_bias)
    if key not in _NC_CACHE:
        _NC_CACHE[key] = _build_bass(with_qkv_bias, with_v_bias)
    nc = _NC_CACHE[key]

    cos2, sin2 = _rope_tables()
    rT = _rot_matrix().astype(BFNP)

    ind = np.zeros((2, 128), dtype=np.float32)
    for f in range(128):
        ind[f // 64, f] = 1.0
    ind = ind.astype(BFNP)

    ones = np.ones((1, 512), dtype=BFNP)

    in_maps = []
    for c in range(N_CORES):
        b = c // 4
        g = c % 4
        q0 = g * HL * HD
        wq = W_qkv[:, q0:q0 + HL * HD]
        wk = W_qkv[:, D + q0:D + q0 + HL * HD]
        wv_flat = W_qkv[:, 2 * D + q0:2 * D + q0 + HL * HD]
        # interleave v columns with a zero (ones-slot) column per head
        wv_aug = np.zeros((D, HL * (HD + 1)), dtype=np.float32)
        bv_aug = np.zeros((1, HL * (HD + 1)), dtype=np.float32)
        for h in range(HL):
            wv_aug[:, h * (HD + 1):h * (HD + 1) + HD] = wv_flat[:, h * HD:(h + 1) * HD]
            bv_aug[0, h * (HD + 1):h * (HD + 1) + HD] = \
                b_qkv[2 * D + q0 + h * HD:2 * D + q0 + (h + 1) * HD]
            bv_aug[0, h * (HD + 1) + HD] = 1.0
        bqk = np.concatenate(
            [b_qkv[q0:q0 + HL * HD], b_qkv[D + q0:D + q0 + HL * HD]]
        ).reshape(1, -1).astype(np.float32)
        amask = np.where(padding_mask[b], np.float32(-1e30), np.float32(0.0))
        amask = np.ascontiguousarray(amask.reshape(T // 128, 128).T.astype(np.float32))
        in_maps.append({
            "xT": np.ascontiguousarray(x[b].T).astype(BFNP),
            "wqk": np.ascontiguousarray(
                np.concatenate([wq, wk], axis=1)).astype(BFNP),
            "wv": wv_aug.astype(BFNP),
            "bqk": bqk.astype(BFNP),
            "bv": bv_aug.astype(BFNP),
            "ones": ones,
            "cos2": cos2,
            "sin2": sin2,
            "rT": rT,
            "ind": ind,
            "amask": amask,
            "wo": np.ascontiguousarray(W_out[q0:q0 + HL * HD, :]).astype(BFNP),
        })

    res = bass_utils.run_bass_kernel_spmd(
        nc, in_maps, core_ids=list(range(N_CORES)), trace=TRACE,
    )
    LAST_RESULTS = res

    out = np.zeros((B, T, D), dtype=np.float32)
    for c in range(N_CORES):
        out[c // 4] += res.results[c]["out_part"].astype(np.float32)
    out += b_out.astype(np.float32)
    return out.astype(np.float32)


# revision 19
# speedup vs baseline: 1.1665x; 1.1665x over previous
"""Multi-head self-attention with RoPE on 8 Trainium2 NeuronCores.

Full inputs in, full output out. Sharding: batch (2) x head-groups (4 heads
per core). Each core computes qkv projections for its heads, RoPE, full
softmax(QK^T)V, and a combined (both head-pairs) partial output projection;
host sums the 4 partials per batch element and adds b_out.

v2: row-tiled score matmuls — each head's QK^T contracts only 64 dims, so
the two heads of a pair run CONCURRENTLY in disjoint PE row-groups
(tile_position (0,0)/(64,0)), halving score PE time. Attention runs in
512-query "eighths" so PSUM fits: score groups [128,1024] (A|B) x2 bufs
(4 banks) + 2 pv accumulators (2 banks) + aux ring (2 banks) = 8 banks.
Emission is software-pipelined per step s: pv(s-1) | score(s+1)/exp(s+1) |
deadline-scheduled micro-units (projections, v, norm, outproj).

Problem shape: B=2, T=2048, D=1024, H=16, HD=64 (hardcoded).
"""

import numpy as np
from contextlib import ExitStack

import ml_dtypes
import concourse.bass as bass
import concourse.mybir as mybir
import concourse.tile as tile
from concourse import bass_utils

B, T, D, H = 2, 2048, 1024, 16
HD = 64          # head dim
HL = 4           # heads per core
N_CORES = 8
ROPE_BASE = 10000.0

F32 = mybir.dt.float32
F32R = mybir.dt.float32r
BF16 = mybir.dt.bfloat16
BFNP = ml_dtypes.bfloat16

Exp = mybir.ActivationFunctionType.Exp

NT = T // 128     # 16 key tiles
NK = D // 128     # 8 contraction chunks
SC = HD ** -0.5
NSTEP = 128       # 2 pairs x 4 eighths x 16 key tiles

# results of the last run (for test harness introspection)
LAST_RESULTS = None
TRACE = False


def _split_excess_waits(nc, cap=1):
    """walrus in this env rejects >1 sync-wait per instruction; split extras
    onto single-wait NoOps on the same engine queue."""
    n = 0
    for f in nc.m.functions:
        for bb in f.blocks:
            insts = bb.instructions
            if not any(
                i.sync_info is not None and len(i.sync_info.on_wait) > cap
                for i in insts
            ):
                continue
            out = []
            for inst in insts:
                si = inst.sync_info
                waits = list(si.on_wait) if si is not None else []
                if len(waits) > cap:
                    extra, keep = waits[:-cap], waits[-cap:]
                    for k, w in enumerate(extra):
                        nop = mybir.InstNoOp(
                            name=f"{inst.name}-ws{k}",
                            engine=inst.engine,
                            sync_info=mybir.SyncInfo(on_wait=[w], on_update=[]),
                            bass_nofuse=True,
                        )
                        nc.register_instruction(nop)
                        out.append(nop)
                        n += 1
                    inst.sync_info = mybir.SyncInfo(
                        on_wait=keep, on_update=list(si.on_update)
                    )
                out.append(inst)
            bb.instructions = out
    return n


def _build_bass(with_qkv_bias, with_v_bias):
    nc = bass.Bass("TRN2", target_bir_lowering=False, debug=False, num_devices=1)

    # ---- DRAM I/O ----
    d_xT = nc.dram_tensor("xT", [D, T], BF16, kind="ExternalInput").ap()
    d_wqk = nc.dram_tensor("wqk", [D, 4 * 128], BF16, kind="ExternalInput").ap()
    d_wv = nc.dram_tensor("wv", [D, HL * (HD + 1)], BF16, kind="ExternalInput").ap()
    d_bqk = nc.dram_tensor("bqk", [1, 4 * 128], BF16, kind="ExternalInput").ap()
    d_bv = nc.dram_tensor("bv", [1, HL * (HD + 1)], BF16, kind="ExternalInput").ap()
    d_ones = nc.dram_tensor("ones", [1, 512], BF16, kind="ExternalInput").ap()
    d_cos = nc.dram_tensor("cos2", [128, T], F32, kind="ExternalInput").ap()
    d_sin = nc.dram_tensor("sin2", [128, T], F32, kind="ExternalInput").ap()
    d_rT = nc.dram_tensor("rT", [128, 128], BF16, kind="ExternalInput").ap()
    d_ind = nc.dram_tensor("ind", [2, 128], BF16, kind="ExternalInput").ap()
    d_amask = nc.dram_tensor("amask", [128, NT], F32, kind="ExternalInput").ap()
    d_wo = nc.dram_tensor("wo", [2 * 128, D], BF16, kind="ExternalInput").ap()
    d_out = nc.dram_tensor("out_part", [T, D], BF16, kind="ExternalOutput").ap()

    with tile.TileContext(nc) as tc, ExitStack() as ctx:
        pool = lambda name, bufs: ctx.enter_context(tc.tile_pool(name=name, bufs=bufs))
        psum = lambda name, bufs: ctx.enter_context(
            tc.tile_pool(name=name, bufs=bufs, space="PSUM")
        )

        p_const = pool("const", 1)
        p_xt = pool("xt", 1)
        p_w = pool("w", 1)
        p_wv = pool("wv", 1)
        p_cs = pool("cs", 1)
        p_tmp = pool("tmp", 2)
        p_qk = pool("qk", 1)
        p_v = pool("v", NT)
        p_e = pool("e", 4)
        p_at = pool("at", 4)
        p_an = pool("an", 2)
        p_fin = pool("fin", 2)

        ps_s = psum("ps_s", 2)      # score groups [128,1024] -> 4 banks
        ps_pv = psum("ps_pv", 1)    # pvA+pvB [65,512] -> 2 banks
        ps_aux = psum("ps_aux", 2)  # [128,512] ring -> 2 banks

        # ---- input loads ----
        # x token-quarter 0 rides the sync ring in per-chunk descriptors
        # interleaved with wqk chunks, so chunk-k matmuls start as soon as
        # pair k lands. Tables split across scalar+vector rings.
        xt_all = p_xt.tile([128, NK * T], BF16, tag="xt", bufs=1, name="xt_all")
        wqk_all = p_w.tile([128, NK * 512], BF16, tag="wqk", bufs=1,
                           name="wqk_all")
        xt3 = xt_all[:].rearrange("p (c w) -> p c w", c=NK)
        xsrc = d_xT[:].rearrange("(c p) w -> p c w", p=128)
        # need-ordered: the q/k weight groups for pair 0 and x token-quarter
        # 0 gate the prologue matmuls; everything else follows.
        wqk3 = wqk_all[:].rearrange("p (c w) -> p c w", c=NK)
        wsrc = d_wqk[:].rearrange("(c p) w -> p c w", p=128)
        with tc.high_priority():
            for g in (0, 2):
                nc.sync.dma_start(wqk3[:, :, g * 128:(g + 1) * 128],
                                  wsrc[:, :, g * 128:(g + 1) * 128])
            nc.sync.dma_start(xt3[:, 0:4, 0:512], xsrc[:, 0:4, 0:512])
            nc.sync.dma_start(xt3[:, 4:8, 0:512], xsrc[:, 4:8, 0:512])
        nc.sync.dma_start(xt3[:, :, 1536:2048], xsrc[:, :, 1536:2048])
        for g in (1, 3):
            nc.sync.dma_start(wqk3[:, :, g * 128:(g + 1) * 128],
                              wsrc[:, :, g * 128:(g + 1) * 128])

        def xt(k):
            return xt_all[:, k * T:(k + 1) * T]

        def wqk_sb(k):
            return wqk_all[:, k * 512:(k + 1) * 512]

        # scalar ring: rope tables / mask / v weights (needed early)
        t_rT = p_const.tile([128, 128], BF16, tag="rT")
        nc.scalar.dma_start(t_rT[:], d_rT[:])
        t_warm = p_const.tile([1, 16], BF16, tag="warmrow")
        nc.scalar.activation(t_warm[:], t_rT[0:1, 0:16], Exp)
        t_cos = p_cs.tile([128, T], F32, tag="cos")
        t_sin = p_cs.tile([128, T], F32, tag="sin")
        nc.gpsimd.dma_start(t_sin[:, 0:1024], d_sin[:, 0:1024])
        nc.gpsimd.dma_start(t_cos[:, 0:1024], d_cos[:, 0:1024])
        nc.gpsimd.dma_start(xt3[:, :, 1024:1536], xsrc[:, :, 1024:1536])
        nc.gpsimd.dma_start(t_sin[:, 1024:T], d_sin[:, 1024:T])
        nc.gpsimd.dma_start(t_cos[:, 1024:T], d_cos[:, 1024:T])
        t_amask = p_const.tile([128, NT], F32, tag="amask")
        nc.scalar.dma_start(t_amask[:], d_amask[:])
        wv_all = p_wv.tile([128, NK * 260], BF16, tag="wv", bufs=1,
                           name="wv_all")
        nc.scalar.dma_start(
            wv_all[:].rearrange("p (c w) -> p c w", c=NK),
            d_wv[:].rearrange("(c p) w -> p c w", p=128),
        )
        nc.scalar.dma_start(xt3[:, :, 512:1024], xsrc[:, :, 512:1024])
        t_ones = p_const.tile([1, 512], BF16, tag="ones")
        nc.scalar.dma_start(t_ones[:], d_ones[:])
        t_bqk = p_const.tile([1, 4 * 128], BF16, tag="bqk")
        nc.scalar.dma_start(t_bqk[:], d_bqk[:])
        t_bv = p_const.tile([1, HL * (HD + 1)], BF16, tag="bv")
        nc.scalar.dma_start(t_bv[:], d_bv[:])

        t_indA = p_const.tile([1, 128], BF16, tag="indA")
        nc.gpsimd.dma_start(t_indA[:], d_ind[0:1, :])
        t_indB = p_const.tile([1, 128], BF16, tag="indB")
        nc.gpsimd.dma_start(t_indB[:], d_ind[1:2, :])

        def wv_sb(k):
            return wv_all[:, k * 260:(k + 1) * 260]

        # out-proj weights: not needed until late; sync ring after x
        wo_sb = []
        for c2 in range(2):
            wt = p_fin.tile([128, D], BF16, tag="wo", name="wo_t")
            nc.sync.dma_start(wt[:], d_wo[c2 * 128:(c2 + 1) * 128, :])
            wo_sb.append(wt)

        # ---- persistent q/k tiles (head A dims on partitions 0:64,
        # head B on 64:128 — no zero padding needed with row tiling) ----
        qc = [p_qk.tile([128, T], BF16, tag=f"qc{p}", name="qc_t")
              for p in range(2)]
        kt = [p_qk.tile([128, T], BF16, tag=f"k{p}", name="k_t")
              for p in range(2)]

        v_sb = [None] * NT
        at_t = [None] * 4   # (pair, hh) -> [65, T] f32 unnormalized attn
        an_t = [None] * 2   # pair -> [128, T] bf16 normalized attn
        for p in range(2):
            an_t[p] = p_an.tile([128, T], BF16, tag="an", name="an_t")

        # ---- rope / projection emitters ----
        def emit_proj_mms(acc, c2, sl):
            for k in range(NK):
                nc.tensor.matmul(
                    acc,
                    wqk_sb(k)[:, c2 * 128:(c2 + 1) * 128],
                    xt(k)[:, sl],
                    start=(k == 0),
                    stop=(not with_qkv_bias and k == NK - 1),
                    skip_group_check=True,
                )
            if with_qkv_bias:
                nc.tensor.matmul(
                    acc,
                    t_bqk[:, c2 * 128:(c2 + 1) * 128],
                    t_ones[:, 0:512],
                    start=False,
                    stop=True,
                    skip_group_check=True,
                )

        def emit_rope(acc, qi, pair, is_k, rot_ring="aux"):
            """RoPE: roped = raw*cos + R @ (raw*sin); store q/k bf16."""
            sl = slice(qi * 512, (qi + 1) * 512)
            u = p_tmp.tile([128, 512], BF16, tag="u", name="u_t")
            nc.vector.tensor_mul(u[:], acc, t_sin[:, sl])
            if rot_ring == "pv":
                rot = ps_pv.tile([128, 512], F32, tag="pvA", name="rot")
            else:
                rot = ps_aux.tile([128, 512], F32, tag="aux", name="rot")
            nc.tensor.matmul(rot[:], t_rT[:], u[:], start=True, stop=True,
                             skip_group_check=True)
            c_sb = p_tmp.tile([128, 512], F32, tag="c", name="c_t")
            nc.vector.tensor_mul(c_sb[:], acc, t_cos[:, sl])
            dst = qc[pair] if not is_k else kt[pair]
            nc.vector.tensor_add(dst[:, sl], c_sb[:], rot[:])

        def emit_v_fin(j, acc):
            av = acc[:, 0:HL * (HD + 1)]
            vt = p_v.tile([128, HL * (HD + 1)], BF16, tag="v", name="v_t")
            nc.vector.tensor_copy(vt[:], av)
            if not with_v_bias:
                ones_cols = vt[:].rearrange("p (h c) -> p h c", h=HL)[:, :, HD:HD + 1]
                nc.gpsimd.memset(ones_cols, 1.0)
            v_sb[j] = vt

        # ---- micro-unit builders (each callable ~1-2 MMs of PE work) ----
        def proj_unit_micros(c2, qi, pair, is_k):
            """Projection quarter as 5 micros: 4x 2-chunk MMs + rope."""
            box = [None]

            def alloc_and_mm(kk):
                def f():
                    if box[0] is None:
                        box[0] = ps_aux.tile([128, 512], F32, tag="aux",
                                             name="acc")
                    acc = box[0]
                    sl = slice(qi * 512, (qi + 1) * 512)
                    for k in (kk, kk + 1):
                        nc.tensor.matmul(
                            acc[:], wqk_sb(k)[:, c2 * 128:(c2 + 1) * 128],
                            xt(k)[:, sl], start=(k == 0),
                            stop=(not with_qkv_bias and k == NK - 1),
                            skip_group_check=True,
                        )
                    if with_qkv_bias and kk == NK - 2:
                        nc.tensor.matmul(
                            acc[:], t_bqk[:, c2 * 128:(c2 + 1) * 128],
                            t_ones[:, 0:512], start=False, stop=True,
                            skip_group_check=True,
                        )
                return f

            def rope_f():
                emit_rope(box[0][:], qi, pair, is_k)

            return [alloc_and_mm(k) for k in range(0, NK, 2)] + [rope_f]

        def v_unit_micros(j):
            """V projection for key tile j as 5 micros."""
            box = [None]

            def mm(kk):
                def f():
                    if box[0] is None:
                        box[0] = ps_aux.tile([128, 512], F32, tag="aux",
                                             name="vacc")
                    av = box[0][:, 0:HL * (HD + 1)]
                    for k in (kk, kk + 1):
                        nc.tensor.matmul(
                            av, xt(k)[:, j * 128:(j + 1) * 128], wv_sb(k)[:],
                            start=(k == 0),
                            stop=(not with_v_bias and k == NK - 1),
                            skip_group_check=True,
                        )
                    if with_v_bias and kk == NK - 2:
                        nc.tensor.matmul(av, t_ones[:, 0:128], t_bv[:],
                                         start=False, stop=True,
                                         skip_group_check=True)
                return f

            def fin():
                emit_v_fin(j, box[0])

            return [mm(k) for k in range(0, NK, 2)] + [fin]

        # ---- normalization ----
        norm_state = {}

        def emit_norm_head(pair, e8):
            """after eighth (pair, e8): denominator reciprocal round-trip.
            Gather the pv PSUM denominator rows into a partition-spread
            [128,8] layout, reciprocal there (vectorized), scatter back to
            flat bf16 rows for the pb broadcast matmuls."""
            sums = p_fin.tile([128, 8], F32, tag="sums", bufs=2, name="sums_t")
            rec = p_fin.tile([128, 8], BF16, tag="rec", bufs=2, name="rec_t")
            rrA = p_fin.tile([1, 512], BF16, tag="rrA", bufs=2, name="rrA_t")
            rrB = p_fin.tile([1, 512], BF16, tag="rrB", bufs=2, name="rrB_t")
            ar = p_fin.tile([128, 512], F32, tag="ar", bufs=2, name="ar_t")
            sl = slice(e8 * 512, (e8 + 1) * 512)
            for hh in range(2):
                nc.sync.dma_start(
                    sums[:, hh * 4:(hh + 1) * 4],
                    at_t[2 * pair + hh][HD:HD + 1, sl].rearrange(
                        "o (p c) -> o p c", p=128))
            with nc.allow_low_precision(reason="1/denominator in bf16"):
                nc.vector.reciprocal(rec[:], sums[:])
            nc.sync.dma_start(rrA[:].rearrange("o (p c) -> o p c", p=128),
                              rec[:, 0:4])
            nc.sync.dma_start(rrB[:].rearrange("o (p c) -> o p c", p=128),
                              rec[:, 4:8])
            norm_state[(pair, e8)] = ((rrA, rrB), ar)

        def emit_ar(pair, e8):
            sl = slice(e8 * 512, (e8 + 1) * 512)
            _, ar = norm_state[(pair, e8)]
            nc.sync.dma_start(ar[HD:128, :], at_t[2 * pair + 1][0:HD, sl])

        def fin_micros(pair, e8):
            """pb broadcast matmuls + an multiplies (lag-scheduled)."""
            sl = slice(e8 * 512, (e8 + 1) * 512)
            box = [None]

            def pb_f():
                (rrA, rrB), ar = norm_state[(pair, e8)]
                pb = ps_aux.tile([128, 512], F32, tag="aux", name="pb")
                nc.tensor.matmul(pb[:], t_indA[:], rrA[:],
                                 start=True, stop=False, skip_group_check=True)
                nc.tensor.matmul(pb[:], t_indB[:], rrB[:],
                                 start=False, stop=True, skip_group_check=True)
                box[0] = pb

            def an_f():
                _, ar = norm_state.pop((pair, e8))
                pb = box[0]
                nc.vector.tensor_mul(an_t[pair][0:HD, sl],
                                     at_t[2 * pair][0:HD, sl], pb[0:HD, :])
                nc.vector.tensor_mul(an_t[pair][HD:128, sl],
                                     ar[HD:128, :], pb[HD:128, :])

            return [pb_f, an_f]

        def outproj_micros(t, tail=False):
            """output projection for token tile t, both pairs accumulated."""
            off = (t % NT) * 128
            osb = p_fin.tile([128, D], BF16, tag="osb", bufs=4, name="osb_t")

            def half(n5):
                def f():
                    s5 = slice(n5 * 512, (n5 + 1) * 512)
                    pp = ps_aux.tile([128, 512], F32, tag="aux", name="pp")
                    nc.tensor.matmul(pp[:], an_t[0][:, off:off + 128],
                                     wo_sb[0][:, s5],
                                     start=True, stop=False,
                                     skip_group_check=True)
                    nc.tensor.matmul(pp[:], an_t[1][:, off:off + 128],
                                     wo_sb[1][:, s5],
                                     start=False, stop=True,
                                     skip_group_check=True)
                    if tail and n5 == 1:
                        nc.scalar.copy(osb[:, s5], pp[:])
                    else:
                        nc.vector.tensor_copy(osb[:, s5], pp[:])
                return f

            def dma_f():
                nc.gpsimd.dma_start(d_out[t * 128:(t + 1) * 128, :], osb[:])

            return [half(0), half(1), dma_f]

        # ---- static micro-unit schedule: step -> [callables] ----
        # NOTE: emission order IS logical program order for the dependency
        # tracker — every unit's final write must be emitted strictly before
        # its first reader. score(s+1) is emitted at the top of step s, and
        # pv(s)/v_sb[jb] at the bottom, which sets the deadlines below.
        sched = [[] for _ in range(NSTEP + 40)]

        def add(step, micros, stride=1):
            s = step
            for m in micros:
                sched[min(max(s, 0), len(sched) - 1)].append(m)
                s += stride

        # v projections: v(j) must be fully emitted before pv(j) at step j
        for j in range(2, NT):
            if j <= 6:
                add(max(0, j - 5), v_unit_micros(j), stride=0)
            else:
                add(j - 6, v_unit_micros(j), stride=1)
        # deadlines: score(s) is emitted at the top of step s-2, so a
        # quarter read from step X must have its rope emitted by step X-3.
        add(0, proj_unit_micros(2, 1, 0, True), stride=0)    # rope @0 <= 1
        add(0, proj_unit_micros(2, 2, 0, True), stride=1)    # rope @5 <= 5
        add(4, proj_unit_micros(2, 3, 0, True), stride=1)    # rope @9 <= 9
        add(8, proj_unit_micros(0, 1, 0, False), stride=1)   # rope @13 <= 13
        add(24, proj_unit_micros(0, 2, 0, False), stride=1)  # rope @29 <= 29
        add(40, proj_unit_micros(0, 3, 0, False), stride=1)  # rope @45 <= 45
        add(28, proj_unit_micros(3, 0, 1, True), stride=2)   # rope @38 <= 61
        add(34, proj_unit_micros(3, 1, 1, True), stride=2)   # rope @44 <= 65
        add(44, proj_unit_micros(3, 2, 1, True), stride=2)   # rope @54 <= 69
        add(50, proj_unit_micros(3, 3, 1, True), stride=2)   # rope @60 <= 73
        add(56, proj_unit_micros(1, 0, 1, False), stride=1)  # rope @61 <= 61
        add(70, proj_unit_micros(1, 1, 1, False), stride=1)  # rope @75 <= 77
        add(86, proj_unit_micros(1, 2, 1, False), stride=1)  # rope @91 <= 93
        add(102, proj_unit_micros(1, 3, 1, False), stride=1) # rope @107 <= 109
        # norm fins: ~4.5us after each eighth ends (DMA round-trip hiding)
        for p in range(2):
            for e8 in range(4):
                add(64 * p + 16 * e8 + 20, fin_micros(p, e8))
        # outproj: tiles 0..7 interleave after fin(1, g); tiles 8..11 fill
        # the tail's norm-chain latency gap; 12..15 after fin(1,3)
        for g in range(2):
            for i in range(4):
                t = 4 * g + i
                add(64 + 16 * g + 23 + i, outproj_micros(t))
        for i in range(4):
            add(NSTEP + i, outproj_micros(8 + i, tail=True))
        for i in range(4):
            add(NSTEP + 8 + i, outproj_micros(12 + i, tail=True))

        # ---- prologue ----
        # warm-up: junk matmuls bridge the DMA wait so HAM reaches 8/8
        # before real work, and a junk activation preloads the exp table.
        warm_ps = ps_aux.tile([128, 512], F32, tag="aux", name="warm_ps")
        for _ in range(55):
            nc.tensor.matmul(warm_ps[:, 0:128], t_rT[:], t_rT[:],
                             start=True, stop=True, skip_group_check=True)
        # k0-q0 + q0-e0 accs in a borrowed score tile
        big = ps_s.tile([128, 1024], F32, tag="s", name="acc_big")
        aK = big[:, 0:512]
        aQ = big[:, 512:1024]
        for k in range(NK):
            last = not with_qkv_bias and k == NK - 1
            nc.tensor.matmul(aK, wqk_sb(k)[:, 2 * 128:3 * 128],
                             xt(k)[:, 0:512], start=(k == 0), stop=last,
                             skip_group_check=True)
            nc.tensor.matmul(aQ, wqk_sb(k)[:, 0:128],
                             xt(k)[:, 0:512], start=(k == 0), stop=last,
                             skip_group_check=True)
        if with_qkv_bias:
            nc.tensor.matmul(aK, t_bqk[:, 2 * 128:3 * 128], t_ones[:, 0:512],
                             start=False, stop=True, skip_group_check=True)
            nc.tensor.matmul(aQ, t_bqk[:, 0:128], t_ones[:, 0:512],
                             start=False, stop=True, skip_group_check=True)
        emit_rope(aK, 0, 0, True, rot_ring="pv")
        emit_rope(aQ, 0, 0, False, rot_ring="pv")
        # v(0), v(1) through the aux ring
        for j in range(2):
            for m in v_unit_micros(j):
                m()

        # ---- attention steps ----
        def step_info(s):
            pair = s // 64
            e8 = (s // 16) % 4
            jb = s % 16
            return pair, e8, jb

        score_of = {}
        e_of = {}
        pv_box = [None, None]

        def emit_score(s):
            pair, e8, jb = step_info(s)
            sl = slice(e8 * 512, (e8 + 1) * 512)
            sAB = ps_s.tile([128, 1024], F32, tag="s", name="sAB")
            ch = slice(jb * 128, (jb + 1) * 128)
            nc.tensor.matmul(sAB[:, 0:512], kt[pair][0:64, ch],
                             qc[pair][0:64, sl], start=True, stop=True,
                             skip_group_check=True, tile_position=(0, 0))
            nc.tensor.matmul(sAB[:, 512:1024], kt[pair][64:128, ch],
                             qc[pair][64:128, sl], start=True, stop=True,
                             skip_group_check=True, tile_position=(64, 0))
            score_of[s] = sAB

        def emit_exp(s):
            pair, e8, jb = step_info(s)
            sAB = score_of.pop(s)
            e = p_e.tile([128, 1024], BF16, tag="e", name="e_t")
            nc.scalar.activation(e[:], sAB[:], Exp,
                                 bias=t_amask[:, jb:jb + 1], scale=SC)
            e_of[s] = e

        def emit_pv(s):
            pair, e8, jb = step_info(s)
            if jb == 0:
                pv_box[0] = ps_pv.tile([HD + 1, 512], F32, tag="pvA",
                                       name="pvA_t")
                pv_box[1] = ps_pv.tile([HD + 1, 512], F32, tag="pvB",
                                       name="pvB_t")
            e = e_of.pop(s)
            h0 = 2 * pair
            nc.tensor.matmul(pv_box[0][:],
                             v_sb[jb][:, h0 * (HD + 1):(h0 + 1) * (HD + 1)],
                             e[:, 0:512], start=(jb == 0), stop=(jb == NT - 1),
                             skip_group_check=True)
            nc.tensor.matmul(pv_box[1][:],
                             v_sb[jb][:, (h0 + 1) * (HD + 1):(h0 + 2) * (HD + 1)],
                             e[:, 512:1024], start=(jb == 0), stop=(jb == NT - 1),
                             skip_group_check=True)

        def emit_atcopy(pair, e8):
            sl = slice(e8 * 512, (e8 + 1) * 512)
            for hh in range(2):
                if at_t[2 * pair + hh] is None:
                    at_t[2 * pair + hh] = p_at.tile([HD + 1, T], F32, tag="aT",
                                                    name="at_t")
            for hh in range(2):
                nc.vector.tensor_copy(at_t[2 * pair + hh][:, sl],
                                      pv_box[hh][:])
            emit_norm_head(pair, e8)
            emit_ar(pair, e8)

        emit_score(0)
        emit_exp(0)
        emit_score(1)
        emit_exp(1)
        for s in range(NSTEP):
            if s + 2 < NSTEP:
                emit_score(s + 2)
                emit_exp(s + 2)
            for m in sched[s]:
                m()
            if s >= 1:
                emit_pv(s - 1)
                pair, e8, jb = step_info(s - 1)
                if jb == NT - 1:
                    emit_atcopy(pair, e8)
        emit_pv(NSTEP - 1)
        emit_atcopy(1, 3)

        # ---- tail: remaining scheduled micros in order ----
        for s in range(NSTEP, len(sched)):
            for m in sched[s]:
                m()

    _split_excess_waits(nc)
    return nc


_NC_CACHE = {}


def _rope_tables():
    inv_freq = (1.0 / (ROPE_BASE ** (np.arange(0, HD, 2, dtype=np.float32) / HD))
                ).astype(np.float32)
    t = np.arange(T, dtype=np.float32)
    freqs = np.einsum("t,f->tf", t, inv_freq).astype(np.float32)  # (T, HD/2)
    emb = np.concatenate([freqs, freqs], axis=-1)                  # (T, HD)
    cosT = np.cos(emb).astype(np.float32).T                        # (HD, T)
    sinT = np.sin(emb).astype(np.float32).T
    cosT = np.ascontiguousarray(np.concatenate([cosT, cosT], axis=0))
    sinT = np.ascontiguousarray(np.concatenate([sinT, sinT], axis=0))
    return cosT, sinT


def _rot_matrix():
    r = np.zeros((128, 128), dtype=np.float32)
    for p0 in (0, 64):
        for d in range(32):
            r[p0 + d, p0 + 32 + d] = -1.0
            r[p0 + 32 + d, p0 + d] = 1.0
    return np.ascontiguousarray(r.T)


def kernel(x, W_qkv, b_qkv, W_out, b_out, padding_mask):
    global _NC_CACHE, LAST_RESULTS
    x = np.asarray(x, dtype=np.float32)
    W_qkv = np.asarray(W_qkv, dtype=np.float32)
    b_qkv = np.asarray(b_qkv, dtype=np.float32)
    W_out = np.asarray(W_out, dtype=np.float32)
    b_out = np.asarray(b_out, dtype=np.float32)
    padding_mask = np.asarray(padding_mask)

    with_qkv_bias = bool(np.any(b_qkv[:2 * D]))
    with_v_bias = bool(np.any(b_qkv[2 * D:]))
    key = (with_qkv_bias, with_v_bias)
    if key not in _NC_CACHE:
        _NC_CACHE[key] = _build_bass(with_qkv_bias, with_v_bias)
    nc = _NC_CACHE[key]

    cos2, sin2 = _rope_tables()
    rT = _rot_matrix().astype(BFNP)

    ind = np.zeros((2, 128), dtype=np.float32)
    for f in range(128):
        ind[f // 64, f] = 1.0
    ind = ind.astype(BFNP)

    ones = np.ones((1, 512), dtype=BFNP)

    in_maps = []
    for c in range(N_CORES):
        b = c // 4
        g = c % 4
        q0 = g * HL * HD
        wq = W_qkv[:, q0:q0 + HL * HD]
        wk = W_qkv[:, D + q0:D + q0 + HL * HD]
        wv_flat = W_qkv[:, 2 * D + q0:2 * D + q0 + HL * HD]
        # interleave v columns with a zero (ones-slot) column per head
        wv_aug = np.zeros((D, HL * (HD + 1)), dtype=np.float32)
        bv_aug = np.zeros((1, HL * (HD + 1)), dtype=np.float32)
        for h in range(HL):
            wv_aug[:, h * (HD + 1):h * (HD + 1) + HD] = wv_flat[:, h * HD:(h + 1) * HD]
            bv_aug[0, h * (HD + 1):h * (HD + 1) + HD] = \
                b_qkv[2 * D + q0 + h * HD:2 * D + q0 + (h + 1) * HD]
            bv_aug[0, h * (HD + 1) + HD] = 1.0
        bqk = np.concatenate(
            [b_qkv[q0:q0 + HL * HD], b_qkv[D + q0:D + q0 + HL * HD]]
        ).reshape(1, -1).astype(np.float32)
        amask = np.where(padding_mask[b], np.float32(-1e30), np.float32(0.0))
        amask = np.ascontiguousarray(amask.reshape(T // 128, 128).T.astype(np.float32))
        in_maps.append({
            "xT": np.ascontiguousarray(x[b].T).astype(BFNP),
            "wqk": np.ascontiguousarray(
                np.concatenate([wq, wk], axis=1)).astype(BFNP),
            "wv": wv_aug.astype(BFNP),
            "bqk": bqk.astype(BFNP),
            "bv": bv_aug.astype(BFNP),
            "ones": ones,
            "cos2": cos2,
            "sin2": sin2,
            "rT": rT,
            "ind": ind,
            "amask": amask,
            "wo": np.ascontiguousarray(W_out[q0:q0 + HL * HD, :]).astype(BFNP),
        })

    res = bass_utils.run_bass_kernel_spmd(
        nc, in_maps, core_ids=list(range(N_CORES)), trace=TRACE,
    )
    LAST_RESULTS = res

    out = np.zeros((B, T, D), dtype=np.float32)
    for c in range(N_CORES):
        out[c // 4] += res.results[c]["out_part"].astype(np.float32)
    out += b_out.astype(np.float32)
    return out.astype(np.float32)
